# revision 9
# baseline (speedup 1.0000x reference)
"""Bass/Tile kernel for nn_AttnModule (sparse_attention).

Reference computation (per batch b):
    scores  = pos_emb @ position_fmap[b].T          # [T, L]
    attn    = softmax(scores, axis=-1)              # softmax over L
    context = attn @ origin_fmap[b]                 # [T, H]
    out     = context @ W_gen.T + b_gen             # [T, C]

Sharding: pure data parallel over batch B=64 -> 8 cores x 8 batches.

Roofline notes (per core): PE stream time is the sum of matmul N-cycles
at 2.4 GHz (LDWEIGHTS hides under the 64-deep reorder window); DMA is
bounded by ~358 GB/s HBM per core. v2 cuts both walls vs the hi/lo
baseline:
  - pos_emb single fp16 term (mm1 8192 -> 4096 cyc/batch); logit
    rounding error from fp16 pf dominates anyway and stays ~1e-2.
  - origin_fmap streamed as fp8 E3M4 (4 mantissa bits, data ~N(0,1)):
    halves the of DMA bytes; mm2 takes the fp8 moving operand at fp16
    rate (mixed-dtype matmul), PSUM accumulates fp32.
  - TP=112 (T=100 padded) instead of 128 trims transpose/mm3 streams.
  - mm3 batched over groups of 4 batches (one [C,4*TP] PSUM tile) to
    amortize the wgT weight load.
  - software-pipelined schedule: front(b) [mm1+softmax+attn-T] runs
    interleaved with back(b-PIPE) [mm2+ctx-T+mm3] and the DMA queue is
    ordered pf0..pf2, (pf3,of0), (pf4,of1).. so arrivals are
    just-in-time for both phases.

Layout choices (host side prep is free):
  - position_fmap shipped pre-transposed per batch: pfT [B, 128, HT, L]
  - pos_emb shipped transposed fp16, T zero-padded to TP: peT [1, H, TP]
  - origin_fmap shipped l-tiled fp8e3: of [B, 128, LT, H]
  - W_gen shipped transposed: wgT [H, C]
  - output produced as [B, C, TP] fp16, transposed back on host.
"""

import numpy as np
import ml_dtypes

import concourse.mybir as mybir
import concourse.tile as tile
from concourse import bacc
from concourse.bass_utils import run_bass_kernel_spmd
from concourse.masks import make_identity

B, L, H, T, C = 64, 1024, 512, 100, 97
TP = 112
NCORES = 8
BPC = B // NCORES  # batches per core

HT = H // 128  # 4 h-tiles
LT = L // 128  # 8 l-tiles

F32 = mybir.dt.float32
AF = mybir.ActivationFunctionType
AX = mybir.AxisListType
OP = mybir.AluOpType

MM_DT = mybir.dt.float16
NP_DT = np.float16
OF_DT = mybir.dt.float8e3
OF_NP = ml_dtypes.float8_e3m4
PE_TERMS = 1
PIPE = 3  # back(b-PIPE) interleaves with front(b)
MM3_GRP = 4


def build_nc(mm_dt=MM_DT, of_dt=OF_DT, pe_terms=PE_TERMS, repeats=1, pipe=PIPE, hw_loop=None):
    """hw_loop=N wraps the body in a For_i hardware loop (timing builds):
    outT becomes an Internal scratch and a tiny token is the only external
    output, so per-call wire traffic is negligible and device time
    dominates."""
    timing = hw_loop is not None
    nc = bacc.Bacc(None, target_bir_lowering=False, debug=False)

    pfT = nc.dram_tensor("pfT", [BPC, 128, HT, L], mm_dt, kind="ExternalInput").ap()
    of = nc.dram_tensor("of", [BPC, 128, LT, H], of_dt, kind="ExternalInput").ap()
    peT = nc.dram_tensor("peT", [pe_terms, H, TP], mm_dt, kind="ExternalInput").ap()
    wgT = nc.dram_tensor("wgT", [H, C], mm_dt, kind="ExternalInput").ap()
    bg = nc.dram_tensor("bg", [C, 1], F32, kind="ExternalInput").ap()
    if timing:
        outT = nc.dram_tensor("outT", [BPC, C, TP], mm_dt, kind="Internal").ap()
        tok = nc.dram_tensor("tok", [C, 1], F32, kind="ExternalOutput").ap()
    else:
        outT = nc.dram_tensor("outT", [BPC, C, TP], mm_dt, kind="ExternalOutput").ap()

    with tile.TileContext(nc) as tc:
        with (
            tc.tile_pool(name="consts", bufs=1) as consts,
            tc.tile_pool(name="pf", bufs=BPC) as pfpool,
            tc.tile_pool(name="ofp", bufs=BPC) as ofpool,
            tc.tile_pool(name="mid", bufs=pipe + 2) as mid,
            tc.tile_pool(name="work", bufs=3) as work,
            tc.tile_pool(name="cts", bufs=2) as ctspool,
            tc.tile_pool(name="ps_scores", bufs=3, space="PSUM") as ps_scores,
            tc.tile_pool(name="ps_tp", bufs=2, space="PSUM") as ps_tp,
            tc.tile_pool(name="ps_ctx", bufs=2, space="PSUM") as ps_ctx,
            tc.tile_pool(name="ps_out", bufs=1, space="PSUM") as ps_out,
        ):
            # ---- constants ----
            peT_sb = consts.tile([128, pe_terms, HT, TP], mm_dt)
            peTr = peT.rearrange("e (ht p) t -> p e ht t", p=128)
            for e in range(pe_terms):
                nc.sync.dma_start(peT_sb[:, e], peTr[:, e])
            wgT_sb = consts.tile([128, HT, C], mm_dt)
            bg_sb = consts.tile([C, 1], F32)
            ident = consts.tile([128, 128], mm_dt)
            make_identity(nc, ident)

            def load_pf(b):
                # per-ht DMAs: separate queue entries; per-partition runs
                # are contiguous 2KB in DRAM
                pf_sb = pfpool.tile([128, HT, L], mm_dt, tag="pf")
                for ht in range(HT):
                    nc.sync.dma_start(pf_sb[:, ht, :], pfT[b, :, ht, :])
                return pf_sb

            def load_of(b):
                of_sb = ofpool.tile([128, LT, H], of_dt, tag="of")
                for i in range(4):
                    nc.sync.dma_start(
                        of_sb[:, 2 * i : 2 * (i + 1), :], of[b, :, 2 * i : 2 * (i + 1), :]
                    )
                return of_sb

            def front_a(b, pf_sb):
                """mm1 + softmax -> (p, rinv). No PE work besides mm1, so the
                PE queue can proceed to an older batch's mm2 while the
                softmax chain (DVE reduce -> ACT exp) drains."""
                sc_ps = [
                    ps_scores.tile([TP, 512], F32, tag="scores", name=f"sc{lh}")
                    for lh in range(L // 512)
                ]
                for lh in range(L // 512):
                    first = True
                    for e in range(pe_terms):
                        for ht in range(HT):
                            last = e == pe_terms - 1 and ht == HT - 1
                            nc.tensor.matmul(
                                sc_ps[lh],
                                lhsT=peT_sb[:, e, ht, :],
                                rhs=pf_sb[:, ht, lh * 512 : (lh + 1) * 512],
                                start=first,
                                stop=last,
                            )
                            first = False

                m2 = work.tile([TP, 2], F32, tag="m2")
                for lh in range(L // 512):
                    nc.vector.tensor_reduce(m2[:, lh : lh + 1], sc_ps[lh], axis=AX.X, op=OP.max)
                negm = work.tile([TP, 1], F32, tag="negm")
                nc.vector.tensor_reduce(negm, m2, axis=AX.X, op=OP.max, negate=True)
                p_sb = work.tile([TP, L], mm_dt, tag="p")
                s2 = work.tile([TP, 2], F32, tag="s2")
                for lh in range(L // 512):
                    nc.scalar.activation(
                        p_sb[:, lh * 512 : (lh + 1) * 512],
                        sc_ps[lh],
                        AF.Exp,
                        bias=negm,
                        scale=1.0,
                        accum_out=s2[:, lh : lh + 1],
                    )
                ssum = work.tile([TP, 1], F32, tag="ssum")
                nc.vector.tensor_reduce(ssum, s2, axis=AX.X, op=OP.add)
                rinv = mid.tile([TP, 1], F32, tag="rinv")
                nc.vector.reciprocal(rinv, ssum)
                return p_sb, rinv

            def front_b(b, p_sb, rinv):
                """attn transpose -> (pT, rinv)."""
                pT_sb = mid.tile([128, LT, TP], mm_dt, tag="pT")
                tp_ps = ps_tp.tile([128, LT, TP], mm_dt, tag="tp")
                for lt in range(LT):
                    nc.tensor.transpose(
                        tp_ps[:, lt, :], p_sb[:, lt * 128 : (lt + 1) * 128], ident[:TP, :TP]
                    )
                half = LT // 2
                nc.vector.tensor_copy(pT_sb[:, :half, :], tp_ps[:, :half, :])
                nc.scalar.copy(pT_sb[:, half:, :], tp_ps[:, half:, :])
                return pT_sb, rinv

            def back_mm2(b, of_sb, pT_sb, rinv):
                """mm2 + rinv-scaled copy out of PSUM."""
                ctx_ps = ps_ctx.tile([TP, H], F32, tag="ctx")
                for lt in range(LT):
                    nc.tensor.matmul(
                        ctx_ps,
                        lhsT=pT_sb[:, lt, :],
                        rhs=of_sb[:, lt, :],
                        start=(lt == 0),
                        stop=(lt == LT - 1),
                    )
                ctx_sb = work.tile([TP, H], mm_dt, tag="ctx_sb")
                nc.vector.tensor_scalar_mul(ctx_sb, ctx_ps[:], rinv)
                return ctx_sb

            def back_tp(b, ctx_sb, cT4_sb):
                """ctx transpose into this group's shared cT tile."""
                j = b % MM3_GRP
                tp_ps = ps_tp.tile([128, HT, TP], mm_dt, tag="tp")
                for ht in range(HT):
                    nc.tensor.transpose(
                        tp_ps[:, ht, :], ctx_sb[:, ht * 128 : (ht + 1) * 128], ident[:TP, :TP]
                    )
                nc.scalar.copy(cT4_sb[:, :, j, :], tp_ps)

            def back_mm3(b0, nb, cT4_sb):
                """mm3 + bias + store for batches b0..b0+nb-1."""
                o_ps = ps_out.tile([C, MM3_GRP * TP], F32, tag="o")
                for ht in range(HT):
                    nc.tensor.matmul(
                        o_ps[:, : nb * TP],
                        lhsT=wgT_sb[:, ht, :],
                        rhs=cT4_sb[:, ht, :nb, :],
                        start=(ht == 0),
                        stop=(ht == HT - 1),
                    )
                out_sb = work.tile([C, MM3_GRP, TP], mm_dt, tag="out_sb")
                nc.vector.tensor_scalar_add(
                    out_sb[:, :nb, :],
                    o_ps[:, : nb * TP].rearrange("c (b t) -> c b t", b=nb),
                    bg_sb,
                )
                nc.gpsimd.dma_start(
                    outT[b0 : b0 + nb].rearrange("b c t -> c b t"), out_sb[:, :nb, :]
                )

            def body(load_consts):
                # ---- DMA queue order: JIT arrivals ----
                pf_tiles = {}
                of_tiles = {}
                for b in range(min(pipe, BPC)):
                    pf_tiles[b] = load_pf(b)
                if load_consts:
                    nc.sync.dma_start(wgT_sb, wgT.rearrange("(ht p) c -> p ht c", p=128))
                    nc.sync.dma_start(bg_sb, bg)
                for b in range(pipe, BPC):
                    pf_tiles[b] = load_pf(b)
                    of_tiles[b - pipe] = load_of(b - pipe)
                for b in range(max(BPC - pipe, 0), BPC):
                    of_tiles[b] = load_of(b)

                # ---- compute: PE order per step i is
                #   mm1(i) | mm2(i-pipe) | attn-T(i) | ctx-T(i-pipe-1) | mm3
                # so the softmax chain of batch i hides under mm2(i-pipe),
                # and every transpose's LDWEIGHTS hides under a long matmul
                # stream via the PE reorder window. ----
                sm = {}
                state = {}
                ctxs = {}
                cT4 = {}

                def do_back(bb):
                    ctxs[bb] = back_mm2(bb, of_tiles.pop(bb), *state.pop(bb))
                    if bb % MM3_GRP == 0:
                        cT4[bb // MM3_GRP] = ctspool.tile(
                            [128, HT, MM3_GRP, TP], mm_dt, tag="cT4", name="cT4"
                        )
                    if bb >= 1:
                        g = (bb - 1) // MM3_GRP
                        back_tp(bb - 1, ctxs.pop(bb - 1), cT4[g])
                        if (bb - 1) % MM3_GRP == MM3_GRP - 1:
                            back_mm3(g * MM3_GRP, MM3_GRP, cT4[g])

                for i in range(BPC + pipe):
                    if i < BPC:
                        sm[i] = front_a(i, pf_tiles.pop(i))
                    if i >= pipe:
                        do_back(i - pipe)
                    if i < BPC:
                        state[i] = front_b(i, *sm.pop(i))
                # drain the last tp/mm3
                g = (BPC - 1) // MM3_GRP
                back_tp(BPC - 1, ctxs.pop(BPC - 1), cT4[g])
                back_mm3(g * MM3_GRP, BPC - g * MM3_GRP, cT4[g])

            if timing:
                nc.sync.dma_start(wgT_sb, wgT.rearrange("(ht p) c -> p ht c", p=128))
                nc.sync.dma_start(bg_sb, bg)
                with tc.For_i(0, hw_loop, 1):
                    body(load_consts=False)
                nc.gpsimd.dma_start(tok, bg_sb)
            else:
                for _rep in range(repeats):
                    body(load_consts=(_rep == 0))

    nc.compile()
    return nc


_NC = None


def _get_nc():
    global _NC
    if _NC is None:
        _NC = build_nc()
    return _NC


def make_in_maps(position_fmap, origin_fmap, pos_emb, W_gen, b_gen, np_dt=NP_DT, pe_terms=PE_TERMS, of_np=OF_NP):
    """Host-side sharding + layout prep. Returns list of per-core input dicts."""
    pf = np.asarray(position_fmap, dtype=np.float32)
    of = np.asarray(origin_fmap, dtype=np.float32)
    pe = np.asarray(pos_emb, dtype=np.float32)
    wg = np.asarray(W_gen, dtype=np.float32)
    bgv = np.asarray(b_gen, dtype=np.float32)

    # [B, L, H] -> [B, H, L] -> [B, 128, HT, L]  (partition-major, h = ht*128 + p)
    pfT = np.ascontiguousarray(
        pf.transpose(0, 2, 1).reshape(B, HT, 128, L).transpose(0, 2, 1, 3)
    ).astype(np_dt)
    # [B, L, H] -> [B, 128, LT, H]  (partition-major tiling, l = lt*128 + p)
    of_c = np.ascontiguousarray(
        of.reshape(B, LT, 128, H).transpose(0, 2, 1, 3)
    ).astype(of_np)

    peT_f32 = np.zeros((H, TP), dtype=np.float32)
    peT_f32[:, :T] = pe.T
    terms = []
    resid = peT_f32
    for _ in range(pe_terms):
        t = resid.astype(np_dt)
        terms.append(t)
        resid = resid - t.astype(np.float32)
    peT = np.ascontiguousarray(np.stack(terms, axis=0))  # [pe_terms, H, TP]

    wgT = np.ascontiguousarray(wg.T).astype(np_dt)
    bg2 = np.ascontiguousarray(bgv.reshape(C, 1)).astype(np.float32)

    in_maps = []
    for i in range(NCORES):
        sl = slice(i * BPC, (i + 1) * BPC)
        in_maps.append(
            {
                "pfT": pfT[sl],
                "of": of_c[sl],
                "peT": peT,
                "wgT": wgT,
                "bg": bg2,
            }
        )
    return in_maps


def kernel(position_fmap, origin_fmap, pos_emb, W_gen, b_gen):
    nc = _get_nc()
    in_maps = make_in_maps(position_fmap, origin_fmap, pos_emb, W_gen, b_gen)
    res = run_bass_kernel_spmd(nc, in_maps, core_ids=list(range(NCORES)))
    outs = [r["outT"] for r in res.results]  # each [BPC, C, TP]
    out = np.concatenate(outs, axis=0)[:, :, :T]  # [B, C, T]
    return np.ascontiguousarray(out.transpose(0, 2, 1)).astype(np.float32)


# revision 37
# speedup vs baseline: 1.4794x; 1.4794x over previous
"""Bass/Tile kernel for nn_AttnModule (sparse_attention).

Reference computation (per batch b):
    scores  = pos_emb @ position_fmap[b].T          # [T, L]
    attn    = softmax(scores, axis=-1)              # softmax over L
    context = attn @ origin_fmap[b]                 # [T, H]
    out     = context @ W_gen.T + b_gen             # [T, C]

Sharding: pure data parallel over batch B=64 -> 8 cores x 8 batches.

Roofline notes (per core): PE stream time is the sum of matmul N-cycles
at 2.4 GHz (LDWEIGHTS hides under the 64-deep reorder window); DMA is
bounded by ~358 GB/s HBM per core. v2 cuts both walls vs the hi/lo
baseline:
  - pos_emb single fp16 term (mm1 8192 -> 4096 cyc/batch); logit
    rounding error from fp16 pf dominates anyway and stays ~1e-2.
  - origin_fmap streamed as fp8 E3M4 (4 mantissa bits, data ~N(0,1)):
    halves the of DMA bytes; mm2 takes the fp8 moving operand at fp16
    rate (mixed-dtype matmul), PSUM accumulates fp32.
  - TP=112 (T=100 padded) instead of 128 trims transpose/mm3 streams.
  - mm3 batched over groups of 4 batches (one [C,4*TP] PSUM tile) to
    amortize the wgT weight load.
  - software-pipelined schedule: front(b) [mm1+softmax+attn-T] runs
    interleaved with back(b-PIPE) [mm2+ctx-T+mm3] and the DMA queue is
    ordered pf0..pf2, (pf3,of0), (pf4,of1).. so arrivals are
    just-in-time for both phases.

Layout choices (host side prep is free):
  - position_fmap shipped pre-transposed per batch: pfT [B, 128, HT, L]
  - pos_emb shipped transposed fp16, T zero-padded to TP: peT [1, H, TP]
  - origin_fmap shipped l-tiled fp8e3: of [B, 128, LT, H]
  - W_gen shipped transposed: wgT [H, C]
  - output produced as [B, C, TP] fp16, transposed back on host.
"""

import numpy as np
import ml_dtypes

import concourse.mybir as mybir
import concourse.tile as tile
from concourse import bacc
from concourse.bass_utils import run_bass_kernel_spmd
from concourse.masks import make_identity

B, L, H, T, C = 64, 1024, 512, 100, 97
TP = 112
NCORES = 8
BPC = B // NCORES  # batches per core

HT = H // 128  # 4 h-tiles
LT = L // 128  # 8 l-tiles

F32 = mybir.dt.float32
AF = mybir.ActivationFunctionType
AX = mybir.AxisListType
OP = mybir.AluOpType

MM_DT = mybir.dt.float16
NP_DT = np.float16
OF_DT = mybir.dt.float8e3
OF_NP = ml_dtypes.float8_e3m4
PE_TERMS = 1
PIPE = 3  # back(b-PIPE) interleaves with front(b)
MM3_GRP = 4


def build_nc(mm_dt=MM_DT, of_dt=OF_DT, pe_terms=PE_TERMS, repeats=1, pipe=PIPE, hw_loop=None, only=None, dma_mode="single", unroll=8, order_swap=False):
    """hw_loop=N wraps the body in a For_i hardware loop (timing builds):
    outT becomes an Internal scratch and a tiny token is the only external
    output, so per-call wire traffic is negligible and device time
    dominates."""
    timing = hw_loop is not None
    nc = bacc.Bacc(None, target_bir_lowering=False, debug=False)

    pfT = nc.dram_tensor("pfT", [BPC, 128, HT, L], mm_dt, kind="ExternalInput").ap()
    of = nc.dram_tensor("of", [BPC, 128, LT, H], of_dt, kind="ExternalInput").ap()
    peT = nc.dram_tensor("peT", [pe_terms, H, TP], mm_dt, kind="ExternalInput").ap()
    wgT = nc.dram_tensor("wgT", [H, C], mm_dt, kind="ExternalInput").ap()
    bg = nc.dram_tensor("bg", [C, 1], F32, kind="ExternalInput").ap()
    if timing:
        outT = nc.dram_tensor("outT", [BPC, C, TP], mm_dt, kind="Internal").ap()
        tok = nc.dram_tensor("tok", [C, 1], F32, kind="ExternalOutput").ap()
    else:
        outT = nc.dram_tensor("outT", [BPC, C, TP], mm_dt, kind="ExternalOutput").ap()

    with tile.TileContext(nc) as tc:
        with (
            tc.tile_pool(name="consts", bufs=1) as consts,
            tc.tile_pool(name="pf", bufs=BPC) as pfpool,
            tc.tile_pool(name="ofp", bufs=BPC) as ofpool,
            tc.tile_pool(name="mid", bufs=pipe + 2) as mid,
            tc.tile_pool(name="work", bufs=3) as work,
            tc.tile_pool(name="cts", bufs=2) as ctspool,
            tc.tile_pool(name="ps_scores", bufs=3, space="PSUM") as ps_scores,
            tc.tile_pool(name="ps_tp", bufs=2, space="PSUM") as ps_tp,
            tc.tile_pool(name="ps_ctx", bufs=2, space="PSUM") as ps_ctx,
            tc.tile_pool(name="ps_out", bufs=1, space="PSUM") as ps_out,
        ):
            # ---- constants ----
            peT_sb = consts.tile([128, pe_terms, HT, TP], mm_dt)
            peTr = peT.rearrange("e (ht p) t -> p e ht t", p=128)
            for e in range(pe_terms):
                nc.sync.dma_start(peT_sb[:, e], peTr[:, e])
            wgT_sb = consts.tile([128, HT, C], mm_dt)
            bg_sb = consts.tile([C, 1], F32)
            ident = consts.tile([128, 128], mm_dt)
            make_identity(nc, ident)

            def load_pf(b):
                pf_sb = pfpool.tile([128, HT, L], mm_dt, tag="pf")
                if dma_mode == "split4":
                    # per-ht DMAs: per-partition runs are contiguous 2KB
                    for ht in range(HT):
                        nc.sync.dma_start(pf_sb[:, ht, :], pfT[b, :, ht, :])
                elif dma_mode in ("single", "2q_big"):
                    # one 1MB dma_start; per-partition run = 8KB contiguous
                    nc.sync.dma_start(pf_sb, pfT[b])
                elif dma_mode == "2q":
                    nc.sync.dma_start(pf_sb[:, :2, :], pfT[b, :, :2, :])
                    nc.scalar.dma_start(pf_sb[:, 2:, :], pfT[b, :, 2:, :])
                elif dma_mode == "3q":
                    nc.sync.dma_start(pf_sb[:, :2, :], pfT[b, :, :2, :])
                    nc.scalar.dma_start(pf_sb[:, 2, :], pfT[b, :, 2, :])
                    nc.gpsimd.dma_start(pf_sb[:, 3, :], pfT[b, :, 3, :])
                return pf_sb

            def load_of(b):
                of_sb = ofpool.tile([128, LT, H], of_dt, tag="of")
                if dma_mode == "split4":
                    for i in range(4):
                        nc.sync.dma_start(
                            of_sb[:, 2 * i : 2 * (i + 1), :], of[b, :, 2 * i : 2 * (i + 1), :]
                        )
                elif dma_mode == "single":
                    nc.sync.dma_start(of_sb, of[b])
                elif dma_mode == "2q_big":
                    nc.scalar.dma_start(of_sb, of[b])
                elif dma_mode == "2q":
                    nc.sync.dma_start(of_sb[:, :4, :], of[b, :, :4, :])
                    nc.scalar.dma_start(of_sb[:, 4:, :], of[b, :, 4:, :])
                elif dma_mode == "3q":
                    nc.scalar.dma_start(of_sb[:, :4, :], of[b, :, :4, :])
                    nc.gpsimd.dma_start(of_sb[:, 4:, :], of[b, :, 4:, :])
                return of_sb

            def front(i, pf_sb, prevT):
                """mm1(i) with attn-T(i-1) transposes interleaved between the
                matmul streams (each transpose's LDWEIGHTS hides under the
                neighboring N=512 stream: ~366ns/pair vs 440 separate), then
                the softmax chain for i on DVE/ACT. Returns (p, rinv) of i;
                stores pT(i-1) into state."""
                tp_ps = None
                if prevT is not None:
                    p_prev, rinv_prev = prevT
                    tp_ps = ps_tp.tile([128, LT, TP], mm_dt, tag="tp", name="tp")
                    pT_sb = mid.tile([128, LT, TP], mm_dt, tag="pT", name="pT")
                k = 0
                if pf_sb is not None:
                    sc_ps = [
                        ps_scores.tile([TP, 512], F32, tag="scores", name=f"sc{lh}")
                        for lh in range(L // 512)
                    ]
                    for lh in range(L // 512):
                        for ht in range(HT):
                            nc.tensor.matmul(
                                sc_ps[lh],
                                lhsT=peT_sb[:, 0, ht, :],
                                rhs=pf_sb[:, ht, lh * 512 : (lh + 1) * 512],
                                start=(ht == 0),
                                stop=(ht == HT - 1),
                            )
                            if tp_ps is not None and k < LT:
                                nc.tensor.transpose(
                                    tp_ps[:, k, :],
                                    p_prev[:, k * 128 : (k + 1) * 128],
                                    ident[:TP, :TP],
                                )
                                k += 1
                if tp_ps is not None:
                    while k < LT:
                        nc.tensor.transpose(
                            tp_ps[:, k, :], p_prev[:, k * 128 : (k + 1) * 128], ident[:TP, :TP]
                        )
                        k += 1
                    half = LT // 2
                    nc.vector.tensor_copy(pT_sb[:, :half, :], tp_ps[:, :half, :])
                    nc.scalar.copy(pT_sb[:, half:, :], tp_ps[:, half:, :])
                    state[i - 1] = (pT_sb, rinv_prev)

                if pf_sb is None:
                    return None
                m2 = work.tile([TP, 2], F32, tag="m2")
                for lh in range(L // 512):
                    nc.vector.tensor_reduce(m2[:, lh : lh + 1], sc_ps[lh], axis=AX.X, op=OP.max)
                negm = work.tile([TP, 1], F32, tag="negm")
                nc.vector.tensor_reduce(negm, m2, axis=AX.X, op=OP.max, negate=True)
                p_sb = work.tile([TP, L], mm_dt, tag="p")
                s2 = work.tile([TP, 2], F32, tag="s2")
                for lh in range(L // 512):
                    nc.scalar.activation(
                        p_sb[:, lh * 512 : (lh + 1) * 512],
                        sc_ps[lh],
                        AF.Exp,
                        bias=negm,
                        scale=1.0,
                        accum_out=s2[:, lh : lh + 1],
                    )
                ssum = work.tile([TP, 1], F32, tag="ssum")
                nc.vector.tensor_reduce(ssum, s2, axis=AX.X, op=OP.add)
                rinv = mid.tile([TP, 1], F32, tag="rinv")
                nc.vector.reciprocal(rinv, ssum)
                return p_sb, rinv

            def back_mm2(bb, of_sb, pT_sb, rinv):
                """mm2(bb) with ctx-T(bb-1) transposes interleaved; then the
                rinv-scaled PSUM copy-out for bb."""
                prev_ctx = ctxs.pop(bb - 1, None)
                tp_ps = None
                if prev_ctx is not None:
                    tp_ps = ps_tp.tile([128, LT, TP], mm_dt, tag="tp", name="tpc")
                k = 0
                ctx_ps = ps_ctx.tile([TP, H], F32, tag="ctx")
                for lt in range(LT):
                    nc.tensor.matmul(
                        ctx_ps,
                        lhsT=pT_sb[:, lt, :],
                        rhs=of_sb[:, lt, :],
                        start=(lt == 0),
                        stop=(lt == LT - 1),
                    )
                    if tp_ps is not None and k < HT:
                        nc.tensor.transpose(
                            tp_ps[:, k, :],
                            prev_ctx[:, k * 128 : (k + 1) * 128],
                            ident[:TP, :TP],
                        )
                        k += 1
                if tp_ps is not None:
                    g = (bb - 1) // MM3_GRP
                    nc.scalar.copy(cT4[g][:, :, (bb - 1) % MM3_GRP, :], tp_ps[:, :HT, :])
                    if (bb - 1) % MM3_GRP == MM3_GRP - 1:
                        back_mm3(g * MM3_GRP, MM3_GRP, cT4[g])
                ctx_sb = work.tile([TP, H], mm_dt, tag="ctx_sb")
                nc.vector.tensor_scalar_mul(ctx_sb, ctx_ps[:], rinv)
                return ctx_sb

            def final_tp(bb):
                """drain: ctx transpose + copy + mm3 for the last group."""
                prev_ctx = ctxs.pop(bb)
                tp_ps = ps_tp.tile([128, LT, TP], mm_dt, tag="tp", name="tpc")
                for k in range(HT):
                    nc.tensor.transpose(
                        tp_ps[:, k, :], prev_ctx[:, k * 128 : (k + 1) * 128], ident[:TP, :TP]
                    )
                g = bb // MM3_GRP
                nc.scalar.copy(cT4[g][:, :, bb % MM3_GRP, :], tp_ps[:, :HT, :])
                back_mm3(g * MM3_GRP, BPC - g * MM3_GRP, cT4[g])

            def back_mm3(b0, nb, cT4_sb):
                """mm3 + bias + store for batches b0..b0+nb-1."""
                o_ps = ps_out.tile([C, MM3_GRP * TP], F32, tag="o")
                for ht in range(HT):
                    nc.tensor.matmul(
                        o_ps[:, : nb * TP],
                        lhsT=wgT_sb[:, ht, :],
                        rhs=cT4_sb[:, ht, :nb, :],
                        start=(ht == 0),
                        stop=(ht == HT - 1),
                    )
                out_sb = work.tile([C, MM3_GRP, TP], mm_dt, tag="out_sb")
                nc.vector.tensor_scalar_add(
                    out_sb[:, :nb, :],
                    o_ps[:, : nb * TP].rearrange("c (b t) -> c b t", b=nb),
                    bg_sb,
                )
                nc.gpsimd.dma_start(
                    outT[b0 : b0 + nb].rearrange("b c t -> c b t"), out_sb[:, :nb, :]
                )

            state = {}
            ctxs = {}
            cT4 = {}

            nodma_tiles = None
            if only == "nodma":
                pf0_sb = consts.tile([128, HT, L], mm_dt, name="pf0c")
                of0_sb = consts.tile([128, LT, H], of_dt, name="of0c")
                nc.sync.dma_start(pf0_sb, pfT[0])
                nc.sync.dma_start(of0_sb, of[0])
                nodma_tiles = (pf0_sb, of0_sb)

            def body(load_consts):
                if only == "empty":
                    nc.vector.tensor_copy(bg_sb, bg_sb)
                    return
                if only == "pe":
                    # dense independent matmuls: 64 x 512 rows = 32768 PE
                    # cycles -> 13.6us warm / 27.3us cold
                    pf_sb = pfpool.tile([128, HT, L], mm_dt, tag="pf", name="pf")
                    nc.sync.dma_start(pf_sb, pfT[0])
                    for i in range(64):
                        sc = ps_scores.tile([TP, 512], F32, tag="scores", name="sc")
                        nc.tensor.matmul(
                            sc,
                            lhsT=peT_sb[:, 0, i % HT, :],
                            rhs=pf_sb[:, i % HT, (i % 2) * 512 : (i % 2 + 1) * 512],
                            start=True,
                            stop=True,
                        )
                    return
                if only == "pe_grp":
                    # 16 groups of 4 accumulating MMs (64 MMs, N=512)
                    pf_sb = pfpool.tile([128, HT, L], mm_dt, tag="pf", name="pf")
                    nc.sync.dma_start(pf_sb, pfT[0])
                    for g in range(16):
                        sc = ps_scores.tile([TP, 512], F32, tag="scores", name="sc")
                        for j in range(4):
                            nc.tensor.matmul(
                                sc,
                                lhsT=peT_sb[:, 0, j, :],
                                rhs=pf_sb[:, j, (g % 2) * 512 : (g % 2 + 1) * 512],
                                start=(j == 0),
                                stop=(j == 3),
                            )
                    return
                if only == "pe_tp":
                    # 64 transposes of [TP,128] -> overhead probe
                    p_sb = work.tile([TP, L], mm_dt, tag="p", name="p")
                    nc.sync.dma_start(p_sb, pfT[0, :TP, 0, :])
                    for i in range(64):
                        tp_ps = ps_tp.tile([128, TP], mm_dt, tag="tp", name="tp")
                        nc.tensor.transpose(
                            tp_ps, p_sb[:, (i % 8) * 128 : (i % 8 + 1) * 128], ident[:TP, :TP]
                        )
                    return
                if only == "pe_same":
                    # 64 MMs N=512, all with the SAME stationary operand
                    pf_sb = pfpool.tile([128, HT, L], mm_dt, tag="pf", name="pf")
                    nc.sync.dma_start(pf_sb, pfT[0])
                    for i in range(64):
                        sc = ps_scores.tile([TP, 512], F32, tag="scores", name="sc")
                        nc.tensor.matmul(
                            sc,
                            lhsT=peT_sb[:, 0, 0, :],
                            rhs=pf_sb[:, i % HT, (i % 2) * 512 : (i % 2 + 1) * 512],
                            start=True,
                            stop=True,
                        )
                    return
                if only == "pe_mix":
                    # 32 MMs N=512 alternating with 32 transposes
                    pf_sb = pfpool.tile([128, HT, L], mm_dt, tag="pf", name="pf")
                    nc.sync.dma_start(pf_sb, pfT[0])
                    p_sb = work.tile([TP, L], mm_dt, tag="p", name="p")
                    nc.sync.dma_start(p_sb, pfT[0, :TP, 0, :])
                    for i in range(32):
                        sc = ps_scores.tile([TP, 512], F32, tag="scores", name="sc")
                        nc.tensor.matmul(
                            sc,
                            lhsT=peT_sb[:, 0, i % HT, :],
                            rhs=pf_sb[:, i % HT, (i % 2) * 512 : (i % 2 + 1) * 512],
                            start=True,
                            stop=True,
                        )
                        tp_ps = ps_tp.tile([128, TP], mm_dt, tag="tp", name="tp")
                        nc.tensor.transpose(
                            tp_ps, p_sb[:, (i % 8) * 128 : (i % 8 + 1) * 128], ident[:TP, :TP]
                        )
                    return
                # ---- DMA queue order: JIT arrivals ----
                pf_tiles = {}
                of_tiles = {}
                if only == "nodma":
                    for b in range(BPC):
                        pf_tiles[b] = nodma_tiles[0]
                        of_tiles[b] = nodma_tiles[1]
                else:
                    for b in range(min(pipe, BPC)):
                        pf_tiles[b] = load_pf(b)
                    if load_consts:
                        nc.sync.dma_start(wgT_sb, wgT.rearrange("(ht p) c -> p ht c", p=128))
                        nc.sync.dma_start(bg_sb, bg)
                    for b in range(pipe, BPC):
                        pf_tiles[b] = load_pf(b)
                        of_tiles[b - pipe] = load_of(b - pipe)
                    for b in range(max(BPC - pipe, 0), BPC):
                        of_tiles[b] = load_of(b)
                if only == "dma":
                    return

                # ---- compute: PE order per step i is
                #   [mm1(i) ⊗ attn-T(i-1)] | [mm2(i-pipe) ⊗ ctx-T(i-pipe-1), mm3]
                # (⊗ = transposes interleaved between matmul streams so their
                # LDWEIGHTS hide; the softmax chain of batch i hides under
                # the back-half matmuls). ----
                state.clear()
                ctxs.clear()
                cT4.clear()
                prev = None
                for i in range(BPC + pipe):
                    if i < BPC:
                        prev = front(i, pf_tiles.pop(i), prev)
                    elif i == BPC:
                        front(i, None, prev)
                        prev = None
                    if i >= pipe:
                        bb = i - pipe
                        if bb % MM3_GRP == 0:
                            cT4[bb // MM3_GRP] = ctspool.tile(
                                [128, HT, MM3_GRP, TP], mm_dt, tag="cT4", name="cT4"
                            )
                        ctxs[bb] = back_mm2(bb, of_tiles.pop(bb), *state.pop(bb))
                final_tp(BPC - 1)

            if timing:
                # unroll several reps per For_i iteration: the loop's
                # all-engine barrier idles PE long enough to re-engage the
                # HAM clock throttle (1.2 GHz), so amortize it to ~1% and
                # measure warm steady-state throughput like an unrolled NEFF
                assert hw_loop % unroll == 0
                nc.sync.dma_start(wgT_sb, wgT.rearrange("(ht p) c -> p ht c", p=128))
                nc.sync.dma_start(bg_sb, bg)
                with tc.For_i(0, hw_loop // unroll, 1):
                    for _u in range(unroll):
                        body(load_consts=False)
                nc.gpsimd.dma_start(tok, bg_sb)
            else:
                for _rep in range(repeats):
                    body(load_consts=(_rep == 0))

    nc.compile()
    return nc


_NC = None


def _get_nc():
    global _NC
    if _NC is None:
        _NC = build_nc()
    return _NC


def make_in_maps(position_fmap, origin_fmap, pos_emb, W_gen, b_gen, np_dt=NP_DT, pe_terms=PE_TERMS, of_np=OF_NP):
    """Host-side sharding + layout prep. Returns list of per-core input dicts."""
    pf = np.asarray(position_fmap, dtype=np.float32)
    of = np.asarray(origin_fmap, dtype=np.float32)
    pe = np.asarray(pos_emb, dtype=np.float32)
    wg = np.asarray(W_gen, dtype=np.float32)
    bgv = np.asarray(b_gen, dtype=np.float32)

    # [B, L, H] -> [B, H, L] -> [B, 128, HT, L]  (partition-major, h = ht*128 + p)
    pfT = np.ascontiguousarray(
        pf.transpose(0, 2, 1).reshape(B, HT, 128, L).transpose(0, 2, 1, 3)
    ).astype(np_dt)
    # [B, L, H] -> [B, 128, LT, H]  (partition-major tiling, l = lt*128 + p)
    of_c = np.ascontiguousarray(
        of.reshape(B, LT, 128, H).transpose(0, 2, 1, 3)
    ).astype(of_np)

    peT_f32 = np.zeros((H, TP), dtype=np.float32)
    peT_f32[:, :T] = pe.T
    terms = []
    resid = peT_f32
    for _ in range(pe_terms):
        t = resid.astype(np_dt)
        terms.append(t)
        resid = resid - t.astype(np.float32)
    peT = np.ascontiguousarray(np.stack(terms, axis=0))  # [pe_terms, H, TP]

    wgT = np.ascontiguousarray(wg.T).astype(np_dt)
    bg2 = np.ascontiguousarray(bgv.reshape(C, 1)).astype(np.float32)

    in_maps = []
    for i in range(NCORES):
        sl = slice(i * BPC, (i + 1) * BPC)
        in_maps.append(
            {
                "pfT": pfT[sl],
                "of": of_c[sl],
                "peT": peT,
                "wgT": wgT,
                "bg": bg2,
            }
        )
    return in_maps


def kernel(position_fmap, origin_fmap, pos_emb, W_gen, b_gen):
    nc = _get_nc()
    in_maps = make_in_maps(position_fmap, origin_fmap, pos_emb, W_gen, b_gen)
    res = run_bass_kernel_spmd(nc, in_maps, core_ids=list(range(NCORES)))
    outs = [r["outT"] for r in res.results]  # each [BPC, C, TP]
    out = np.concatenate(outs, axis=0)[:, :, :T]  # [B, C, T]
    return np.ascontiguousarray(out.transpose(0, 2, 1)).astype(np.float32)
